# revision 6
# baseline (speedup 1.0000x reference)
"""Trainium2 Bass kernel for nn_CovarianceLayer (Toeplitz-autocorrelation form).

Math: x = inputs[:,0,:] + i*inputs[:,1,:]  (B=256 complex signals, N=1024)
      cov[b,l,m] = Re(hankel @ hankel^H)[l,m] / L  with hankel[b,i,j] = x[b,(j+i)%N]
By circularity cov[b,l,m] = r_b[|l-m|] / L where
      r_b[d] = sum_n ( xr[n]xr[n+d] + xi[n]xi[n+d] )   (indices mod N)
i.e. each [L,L] output tile is a symmetric Toeplitz matrix built from a
128-point autocorrelation.

Per-core plan (32 batches/core, pure data parallel):
  - 2 gpsimd casting DMAs build a wrap-padded fp8 copy of x in DRAM
    (xdup row per batch: [x0|wrap|x1|wrap], 2*1152 elems).
  - per 8-batch group: 2 DMAs build a packed Hankel tile
    Hg[(16c+p), j*1136+u] = x_c[b_j, p+u]; 32 DoubleRow fp8 matmuls per
    batch (K=32 contracts comps+offsets) accumulate r_b into psum col j.
  - drain+1/L on DVE, PE-transpose, palindrome copy s[b,k]=r_b[|k-127|]
    into an SBUF row per batch, then ONE strided DMA per group expands
    the Toeplitz tiles straight from SBUF into the output:
    out[b,l,m] = s[b, 127-l+m]  (contiguous 512B runs both sides).
All Hankel DMAs are issued up front (4-deep buffers) so transfers,
matmuls, and expansion DMAs of different groups overlap.
"""

import numpy as np

import concourse.bacc as bacc
import concourse.mybir as mybir
import concourse.tile as tile
from concourse.bass_types import AP
from concourse.bass_utils import run_bass_kernel_spmd

B, L, N = 256, 128, 1024
NCORES = 8
BPC = B // NCORES  # 32 batches per core

P = 16  # n-offsets per matmul chunk
K = 2 * P  # contraction width (comps folded)
T = N // P  # 64 chunks per batch -> 32 DoubleRow matmuls
W = N - P + 128  # 1136: hankel window elems per partition per batch
CROW = 1152  # padded per-comp row in xdup
ROW = 2 * CROW  # 2304: xdup elems per batch
GB = 8  # batches per pipeline group
NG = BPC // GB  # 4 groups

_CACHE = {}
LAST_RESULT = None


def build_nc():
    f8 = mybir.dt.float8e4
    f32 = mybir.dt.float32
    nc = bacc.Bacc(
        "TRN2", target_bir_lowering=False, debug=False, num_devices=NCORES
    )
    inp = nc.dram_tensor("inp", [BPC, 2, N], f32, kind="ExternalInput")
    out = nc.dram_tensor("out", [BPC, L, L], f32, kind="ExternalOutput")

    with tile.TileContext(nc) as tc:
        with (
            tc.tile_pool(name="const", bufs=1) as cpool,
            tc.tile_pool(name="dram", bufs=1, space="DRAM") as dpool,
            tc.tile_pool(name="hank", bufs=NG) as hpool,
            tc.tile_pool(name="spal", bufs=NG) as spool,
            tc.tile_pool(name="rr", bufs=NG) as rpool,
            tc.tile_pool(name="psum", bufs=NG, space="PSUM") as ppool,
            tc.tile_pool(name="pst", bufs=NG, space="PSUM") as tpool,
        ):
            # --- identity for PE transpose (overlaps with DMAs) ---
            ones = cpool.tile([128, 128], f32)
            nc.vector.memset(ones[:], 1.0)
            ident = cpool.tile([128, 128], f32)
            nc.gpsimd.affine_select(
                out=ident[:],
                in_=ones[:],
                pattern=[[1, 128]],
                compare_op=mybir.AluOpType.is_equal,
                fill=0.0,
                base=0,
                channel_multiplier=-1,
            )

            # --- wrap-padded fp8 signal in DRAM via casting DMAs ---
            xdup = dpool.tile([2 * BPC, CROW], f8)  # row (2b+c) = x_c[b] padded
            flat = inp[:].rearrange("b c n -> (b c) n")
            nc.gpsimd.dma_start(out=xdup[:, 0:N], in_=flat)
            nc.gpsimd.dma_start(out=xdup[:, N:CROW], in_=flat[:, 0:128])

            # --- all hankel tile DMAs first: Hg[16c+p, j*W+u] = x_c[b_j, p+u]
            hgs = []
            for g in range(NG):
                hg = hpool.tile([K, GB * W], f8)
                hgs.append(hg)
                for c in range(2):
                    src = AP(
                        tensor=xdup.tensor,
                        offset=xdup.offset + g * GB * ROW + c * CROW,
                        ap=[[1, P], [ROW, GB], [1, W]],
                    )
                    eng = nc.sync if (2 * g + c) % 2 == 0 else nc.scalar
                    eng.dma_start(out=hg[16 * c : 16 * c + 16, :], in_=src)

            rgs = {}

            def finish(g):
                # transpose r columns -> [GB, 128] rows, palindrome, expand
                pt = tpool.tile([GB, 128], f32)
                nc.tensor.transpose(pt[:], rgs[g][:], ident[:])
                rows = spool.tile([GB, 256], f32)
                nc.scalar.mul(rows[:, 127:255], pt[:], 1.0)
                nc.vector.tensor_copy(rows[:, 0:127], pt[:, 127:0:-1])
                src2 = AP(
                    tensor=rows.tensor,
                    offset=rows.offset + 127,
                    ap=[[256, GB], [-1, 128], [1, 128]],
                )
                dst2 = AP(
                    tensor=out,
                    offset=g * GB * L * L,
                    ap=[[L * L, GB], [L, 128], [1, 128]],
                )
                eng = nc.sync if g % 2 == 0 else nc.scalar
                eng.dma_start(out=dst2, in_=src2)

            for g in range(NG):
                hg = hgs[g]
                # --- autocorrelation matmuls: psum col j accumulates r_b ---
                ps = ppool.tile([128, GB], f32)
                for j in range(GB):
                    col = j * W
                    for tp in range(T // 2):
                        off = col + K * tp  # = 16*(2*tp)
                        lhsT = AP(
                            tensor=hg.tensor,
                            offset=hg.offset + off,
                            ap=[[GB * W, K], [P, 2], [1, 128]],
                        )
                        rhs = AP(
                            tensor=hg.tensor,
                            offset=hg.offset + off,
                            ap=[[GB * W, K], [P, 2], [1, 1]],
                        )
                        nc.tensor.matmul(
                            ps[:, j : j + 1],
                            lhsT,
                            rhs,
                            start=(tp == 0),
                            stop=(tp == T // 2 - 1),
                            perf_mode=mybir.MatmulPerfMode.DoubleRow,
                        )

                # --- drain + 1/L on DVE (PE moves on to next group) ---
                rg = rpool.tile([128, GB], f32)
                nc.vector.tensor_scalar_mul(rg[:], ps[:], 1.0 / L)
                rgs[g] = rg
                # finish previous group now: its drain is done by the time
                # this group's matmuls retire, so the transpose never stalls PE
                if g >= 1:
                    finish(g - 1)
            finish(NG - 1)

    nc.compile()
    return nc


def kernel(inputs: np.ndarray) -> np.ndarray:
    global LAST_RESULT
    inputs = np.ascontiguousarray(np.asarray(inputs), dtype=np.float32)
    assert inputs.shape == (B, 2, N), inputs.shape

    if "nc" not in _CACHE:
        _CACHE["nc"] = build_nc()
    nc = _CACHE["nc"]

    in_maps = [{"inp": inputs[c * BPC : (c + 1) * BPC]} for c in range(NCORES)]
    res = run_bass_kernel_spmd(nc, in_maps, list(range(NCORES)), trace=False)
    LAST_RESULT = res
    outf = np.concatenate([res.results[c]["out"] for c in range(NCORES)], axis=0)
    return outf.reshape(B, L, L, 1).astype(np.float32, copy=False)


# revision 18
# speedup vs baseline: 1.2133x; 1.2133x over previous
"""Trainium2 Bass kernel for nn_CovarianceLayer (Toeplitz-autocorrelation form).

Math: x = inputs[:,0,:] + i*inputs[:,1,:]  (B=256 complex signals, N=1024)
      cov[b,l,m] = Re(hankel @ hankel^H)[l,m] / L  with hankel[b,i,j] = x[b,(j+i)%N]
By circularity cov[b,l,m] = r_b[|l-m|] / L where
      r_b[d] = sum_n ( xr[n]xr[n+d] + xi[n]xi[n+d] )   (indices mod N)
i.e. each [L,L] output tile is a symmetric Toeplitz matrix built from a
128-point autocorrelation, so only r_b (128 values) is ever computed.

Per-core plan (32 batches/core, pure data parallel):
  - gpsimd casting DMAs build a doubled fp8 copy of x in DRAM
    (xdup row per batch: [x0 x0 | x1 x1]; doubling = circular wrap pad).
    Split so the first two batches are ready early.
  - per batch supergroup: 2 DMAs build a packed Hankel tile
    H[16c+p, j*W+u] = x_c[b_j, p+u]; 32 DoubleRow fp8 matmuls per batch
    (K=32 contracts comps+offsets, 2 k-tiles per instruction) accumulate
    r_b into a psum column.
  - drain+1/L on DVE, PE-transpose, palindrome copy s[b,k]=r_b[|k-127|]
    into an SBUF row per batch, then one strided DMA per group expands
    the Toeplitz tiles straight from SBUF into the output:
    out[b,l,m] = s[b, 127-l+m]  (contiguous 512B runs both sides).
Group sizes taper small->large->small so the first expansion DMA starts
early (DMA engines saturate sooner) and the last group's drain->expand
tail is short.
"""

import numpy as np

import concourse.bacc as bacc
import concourse.mybir as mybir
import concourse.tile as tile
from concourse.bass_types import AP
from concourse.bass_utils import run_bass_kernel_spmd

B, L, N = 256, 128, 1024
NCORES = 8
BPC = B // NCORES  # 32 batches per core

P = 16  # n-offsets per chunk
K = 2 * P  # matmul contraction width
T = N // P  # 64 chunks per batch
W = N - P + 128  # 1136
CROW = 2 * N  # doubled per-comp row in xdup
ROW = 2 * CROW  # 4096 elems per batch

_CACHE = {}
LAST_RESULT = None

SGROUPS = [8, 8, 8, 8]  # hankel supergroup sizes
GROUPS = [4, 4, 4, 4, 4, 4, 4, 4]  # compute/expansion group sizes
CAST_SPLIT = 0  # batches cast in the first (fast) casting DMA; 0 = single


def build_nc(sgroups=None, groups=None, cast_split=CAST_SPLIT, act_copy=False,
             warmup=False, ident_input=True, last_sp=0, sp_only=False):
    f8 = mybir.dt.float8e4
    f32 = mybir.dt.float32
    sgroups = list(SGROUPS if sgroups is None else sgroups)
    groups = list(GROUPS if groups is None else groups)
    assert sum(sgroups) == BPC and sum(groups) == BPC
    sstart = [sum(sgroups[:i]) for i in range(len(sgroups))]
    gstart = [sum(groups[:i]) for i in range(len(groups))]
    ng = len(groups)
    # map batch -> supergroup index
    b2s = []
    for s, sz in enumerate(sgroups):
        b2s += [s] * sz

    nc = bacc.Bacc(
        "TRN2", target_bir_lowering=False, debug=False, num_devices=NCORES
    )
    inp = nc.dram_tensor("inp", [BPC, 2, N], f32, kind="ExternalInput")
    identd = (
        nc.dram_tensor("identd", [128, 128], f32, kind="ExternalInput")
        if ident_input
        else None
    )
    out = nc.dram_tensor("out", [BPC, L, L], f32, kind="ExternalOutput")

    with tile.TileContext(nc) as tc:
        with (
            tc.tile_pool(name="const", bufs=1) as cpool,
            tc.tile_pool(name="dram", bufs=1, space="DRAM") as dpool,
            tc.tile_pool(name="hank", bufs=len(sgroups)) as hpool,
            tc.tile_pool(name="spal", bufs=ng) as spool,
            tc.tile_pool(name="rr", bufs=ng) as rpool,
            tc.tile_pool(name="psum", bufs=4, space="PSUM") as ppool,
            tc.tile_pool(name="pst", bufs=3, space="PSUM") as tpool,
        ):
            # --- doubled fp8 signal in DRAM via per-supergroup casting
            # DMAs, emitted FIRST. Each supergroup gets its own DRAM tile so
            # its hankel DMAs depend only on its own cast (tile-granular
            # dependency tracking). ---
            flat = inp[:].rearrange("b c n -> (b c) n")
            xdups = []
            for si, ssz in enumerate(sgroups):
                xd = dpool.tile([2 * ssz, CROW], f8)
                xdups.append(xd)
                r0 = 2 * sstart[si]
                src0 = AP(
                    tensor=flat.tensor,
                    offset=flat.offset + r0 * N,
                    ap=[[N, 2 * ssz], [0, 2], [1, N]],
                )
                dst0 = AP(
                    tensor=xd.tensor,
                    offset=xd.offset,
                    ap=[[CROW, 2 * ssz], [N, 2], [1, N]],
                )
                nc.gpsimd.dma_start(out=dst0, in_=src0)

            # --- identity for PE transpose (needed only ~9us in) ---
            ident = cpool.tile([128, 128], f32)
            if ident_input:
                (nc.sync if sp_only else nc.scalar).dma_start(
                    out=ident[:], in_=identd[:]
                )
            else:
                ones = cpool.tile([128, 128], f32)
                nc.vector.memset(ones[:], 1.0)
                nc.gpsimd.affine_select(
                    out=ident[:],
                    in_=ones[:],
                    pattern=[[1, 128]],
                    compare_op=mybir.AluOpType.is_equal,
                    fill=0.0,
                    base=0,
                    channel_multiplier=-1,
                )
            if warmup:
                warm = cpool.tile([1, 2], f32)
                nc.vector.memset(warm[:], 1.0)
                nc.scalar.mul(warm[:], warm[:], 1.0)

            # --- hankel tiles: H[16c+p, j*W+u] = x_c[b_j, p+u] ---
            htiles = []
            for s, ssz in enumerate(sgroups):
                ht = hpool.tile([K, ssz * W], f8)
                htiles.append(ht)
                xd = xdups[s]
                if ssz == 1:
                    src = AP(
                        tensor=xd.tensor,
                        offset=xd.offset,
                        ap=[[CROW, 2], [1, P], [1, W]],
                    )
                    eng = nc.sync if (sp_only or s % 2 == 0) else nc.scalar
                    eng.dma_start(out=ht[:], in_=src)
                else:
                    for c in range(2):
                        src = AP(
                            tensor=xd.tensor,
                            offset=xd.offset + c * CROW,
                            ap=[[1, P], [ROW, ssz], [1, W]],
                        )
                        eng = nc.sync if (sp_only or c == 0) else nc.scalar
                        eng.dma_start(out=ht[P * c : P * c + P, :], in_=src)

            rgs = {}

            def finish(g):
                gb = groups[g]
                pt = tpool.tile([gb, 128], f32)
                nc.tensor.transpose(pt[:], rgs[g][:], ident[:])
                rows = spool.tile([gb, 256], f32)
                # two palindrome halves; near the tail, split across Act+DVE
                # (Act's sequencer is free of DMA issue by then)
                if act_copy or sp_only:
                    nc.scalar.mul(rows[:, 127:255], pt[:], 1.0)
                else:
                    nc.vector.tensor_copy(rows[:, 127:255], pt[:])
                nc.vector.tensor_copy(rows[:, 0:127], pt[:, 127:0:-1])
                src2 = AP(
                    tensor=rows.tensor,
                    offset=rows.offset + 127,
                    ap=[[256, gb], [-1, 128], [1, 128]],
                )
                dst2 = AP(
                    tensor=out,
                    offset=gstart[g] * L * L,
                    ap=[[L * L, gb], [L, 128], [1, 128]],
                )
                if sp_only or g >= ng - last_sp:
                    eng = nc.sync
                else:
                    eng = nc.sync if g % 2 == 0 else nc.scalar
                eng.dma_start(out=dst2, in_=src2)

            for g in range(ng):
                gb = groups[g]
                ps = ppool.tile([128, gb], f32)
                fin_at = max(1, gb // 2)  # finish prev group mid-way through
                for j in range(gb):
                    if j == fin_at and g >= 1:
                        finish(g - 1)
                    b = gstart[g] + j
                    s = b2s[b]
                    ht = htiles[s]
                    col = (b - sstart[s]) * W
                    for tp in range(T // 2):
                        off = col + K * tp
                        lhsT = AP(
                            tensor=ht.tensor,
                            offset=ht.offset + off,
                            ap=[[sgroups[s] * W, K], [P, 2], [1, 128]],
                        )
                        rhs = AP(
                            tensor=ht.tensor,
                            offset=ht.offset + off,
                            ap=[[sgroups[s] * W, K], [P, 2], [1, 1]],
                        )
                        nc.tensor.matmul(
                            ps[:, j : j + 1],
                            lhsT,
                            rhs,
                            start=(tp == 0),
                            stop=(tp == T // 2 - 1),
                            perf_mode=mybir.MatmulPerfMode.DoubleRow,
                        )

                rg = rpool.tile([128, gb], f32)
                nc.vector.tensor_scalar_mul(rg[:], ps[:], 1.0 / L)
                rgs[g] = rg
            finish(ng - 1)

    nc.compile()
    return nc


def kernel(inputs: np.ndarray) -> np.ndarray:
    global LAST_RESULT
    inputs = np.ascontiguousarray(np.asarray(inputs), dtype=np.float32)
    assert inputs.shape == (B, 2, N), inputs.shape

    if "nc" not in _CACHE:
        _CACHE["nc"] = build_nc()
    nc = _CACHE["nc"]

    ident = np.eye(128, dtype=np.float32)
    in_maps = [
        {"inp": inputs[c * BPC : (c + 1) * BPC], "identd": ident}
        for c in range(NCORES)
    ]
    res = run_bass_kernel_spmd(nc, in_maps, list(range(NCORES)), trace=False)
    LAST_RESULT = res
    outf = np.concatenate([res.results[c]["out"] for c in range(NCORES)], axis=0)
    return outf.reshape(B, L, L, 1).astype(np.float32, copy=False)


# revision 23
# speedup vs baseline: 1.2361x; 1.0188x over previous
"""Trainium2 Bass kernel for nn_CovarianceLayer (Toeplitz-autocorrelation form).

Math: x = inputs[:,0,:] + i*inputs[:,1,:]  (B=256 complex signals, N=1024)
      cov[b,l,m] = Re(hankel @ hankel^H)[l,m] / L  with hankel[b,i,j] = x[b,(j+i)%N]
By circularity cov[b,l,m] = r_b[|l-m|] / L where
      r_b[d] = sum_n ( xr[n]xr[n+d] + xi[n]xi[n+d] )   (indices mod N)
i.e. each [L,L] output tile is a symmetric Toeplitz matrix built from a
128-point autocorrelation, so only r_b (128 values) is ever computed.

Per-core plan (32 batches/core, pure data parallel):
  - gpsimd casting DMAs build a doubled fp8 copy of x in DRAM
    (xdup row per batch: [x0 x0 | x1 x1]; doubling = circular wrap pad).
    Split so the first two batches are ready early.
  - per batch supergroup: 2 DMAs build a packed Hankel tile
    H[16c+p, j*W+u] = x_c[b_j, p+u]; 32 DoubleRow fp8 matmuls per batch
    (K=32 contracts comps+offsets, 2 k-tiles per instruction) accumulate
    r_b into a psum column.
  - drain+1/L on DVE, PE-transpose, palindrome copy s[b,k]=r_b[|k-127|]
    into an SBUF row per batch, then one strided DMA per group expands
    the Toeplitz tiles straight from SBUF into the output:
    out[b,l,m] = s[b, 127-l+m]  (contiguous 512B runs both sides).
Group sizes taper small->large->small so the first expansion DMA starts
early (DMA engines saturate sooner) and the last group's drain->expand
tail is short.
"""

import numpy as np

import concourse.bacc as bacc
import concourse.mybir as mybir
import concourse.tile as tile
from concourse.bass_types import AP
from concourse.bass_utils import run_bass_kernel_spmd

B, L, N = 256, 128, 1024
NCORES = 8
BPC = B // NCORES  # 32 batches per core

P = 16  # n-offsets per chunk
K = 2 * P  # matmul contraction width
T = N // P  # 64 chunks per batch
W = N - P + 128  # 1136
CROW = 2 * N  # doubled per-comp row in xdup
ROW = 2 * CROW  # 4096 elems per batch

_CACHE = {}
LAST_RESULT = None

SGROUPS = [8, 8, 8, 8]  # hankel supergroup sizes
GROUPS = [4, 4, 4, 4, 4, 4, 4, 4]  # compute/expansion group sizes
CAST_SPLIT = 0  # batches cast in the first (fast) casting DMA; 0 = single


def build_nc(sgroups=None, groups=None, cast_split=CAST_SPLIT, act_copy=False,
             warmup=False, ident_input=True, last_sp=0, sp_only=False,
             pal_mm="f32r", cgroups=None, fin_at=None):
    f8 = mybir.dt.float8e4
    f32 = mybir.dt.float32
    sgroups = list(SGROUPS if sgroups is None else sgroups)
    groups = list(GROUPS if groups is None else groups)
    assert sum(sgroups) == BPC and sum(groups) == BPC
    sstart = [sum(sgroups[:i]) for i in range(len(sgroups))]
    gstart = [sum(groups[:i]) for i in range(len(groups))]
    ng = len(groups)
    # map batch -> supergroup index
    b2s = []
    for s, sz in enumerate(sgroups):
        b2s += [s] * sz

    nc = bacc.Bacc(
        "TRN2", target_bir_lowering=False, debug=False, num_devices=NCORES
    )
    inp = nc.dram_tensor("inp", [BPC, 2, N], f32, kind="ExternalInput")
    pal_dt = {None: None, "f32": f32, "f32r": mybir.dt.float32r}[pal_mm]
    identd = (
        nc.dram_tensor("identd", [128, 128], f32, kind="ExternalInput")
        if ident_input and not pal_mm
        else None
    )
    spald = (
        nc.dram_tensor("spald", [128, 256], pal_dt, kind="ExternalInput")
        if pal_mm
        else None
    )
    out = nc.dram_tensor("out", [BPC, L, L], f32, kind="ExternalOutput")

    with tile.TileContext(nc) as tc:
        with (
            tc.tile_pool(name="const", bufs=1) as cpool,
            tc.tile_pool(name="dram", bufs=1, space="DRAM") as dpool,
            tc.tile_pool(name="hank", bufs=len(sgroups)) as hpool,
            tc.tile_pool(name="spal", bufs=ng) as spool,
            tc.tile_pool(name="rr", bufs=ng) as rpool,
            tc.tile_pool(name="psum", bufs=4, space="PSUM") as ppool,
            tc.tile_pool(name="pst", bufs=3, space="PSUM") as tpool,
        ):
            # --- doubled fp8 signal in DRAM via per-supergroup casting
            # DMAs, emitted FIRST. Each supergroup gets its own DRAM tile so
            # its hankel DMAs depend only on its own cast (tile-granular
            # dependency tracking). ---
            flat = inp[:].rearrange("b c n -> (b c) n")
            cgs = list(cgroups) if cgroups else list(sgroups)
            assert sum(cgs) == BPC
            cstart = [sum(cgs[:i]) for i in range(len(cgs))]
            xdups = []  # one per cast group
            for si, ssz in enumerate(cgs):
                xd = dpool.tile([2 * ssz, CROW], f8)
                xdups.append(xd)
                r0 = 2 * cstart[si]
                src0 = AP(
                    tensor=flat.tensor,
                    offset=flat.offset + r0 * N,
                    ap=[[N, 2 * ssz], [0, 2], [1, N]],
                )
                dst0 = AP(
                    tensor=xd.tensor,
                    offset=xd.offset,
                    ap=[[CROW, 2 * ssz], [N, 2], [1, N]],
                )
                nc.gpsimd.dma_start(out=dst0, in_=src0)

            # --- identity for PE transpose (needed only ~9us in) ---
            if pal_mm:
                spal_t = cpool.tile([128, 256], pal_dt)
                (nc.sync if sp_only else nc.scalar).dma_start(
                    out=spal_t[:], in_=spald[:]
                )
                ident = None
            else:
                ident = cpool.tile([128, 128], f32)
                if ident_input:
                    (nc.sync if sp_only else nc.scalar).dma_start(
                        out=ident[:], in_=identd[:]
                    )
                else:
                    ones = cpool.tile([128, 128], f32)
                    nc.vector.memset(ones[:], 1.0)
                    nc.gpsimd.affine_select(
                        out=ident[:],
                        in_=ones[:],
                        pattern=[[1, 128]],
                        compare_op=mybir.AluOpType.is_equal,
                        fill=0.0,
                        base=0,
                        channel_multiplier=-1,
                    )
            if warmup:
                warm = cpool.tile([1, 2], f32)
                nc.vector.memset(warm[:], 1.0)
                nc.scalar.mul(warm[:], warm[:], 1.0)

            # --- hankel tiles: H[16c+p, j*W+u] = x_c[b_j, p+u] ---
            htiles = []
            for s, ssz in enumerate(sgroups):
                ht = hpool.tile([K, ssz * W], f8)
                htiles.append(ht)
                ci = max(i for i in range(len(cgs)) if cstart[i] <= sstart[s])
                assert cstart[ci] + cgs[ci] >= sstart[s] + ssz, "sg spans casts"
                xd = xdups[ci]
                xoff = 2 * (sstart[s] - cstart[ci]) * CROW
                if ssz == 1:
                    src = AP(
                        tensor=xd.tensor,
                        offset=xd.offset + xoff,
                        ap=[[CROW, 2], [1, P], [1, W]],
                    )
                    eng = nc.sync if (sp_only or s % 2 == 0) else nc.scalar
                    eng.dma_start(out=ht[:], in_=src)
                else:
                    for c in range(2):
                        src = AP(
                            tensor=xd.tensor,
                            offset=xd.offset + xoff + c * CROW,
                            ap=[[1, P], [ROW, ssz], [1, W]],
                        )
                        eng = nc.sync if (sp_only or c == 0) else nc.scalar
                        eng.dma_start(out=ht[P * c : P * c + P, :], in_=src)

            rgs = {}

            def finish(g):
                gb = groups[g]
                rows = spool.tile([gb, 256], f32)
                if pal_mm:
                    # one matmul applies transpose+mirror+1/L:
                    # pt2[j,k] = sum_d rg[d,j]*spal[d,k],  spal[d,k]=[d==|k-127|]/L
                    pt2 = tpool.tile([gb, 256], f32)
                    nc.tensor.matmul(pt2[:], rgs[g][:], spal_t[:])
                    nc.vector.tensor_copy(rows[:, 0:255], pt2[:, 0:255])
                else:
                    pt = tpool.tile([gb, 128], f32)
                    nc.tensor.transpose(pt[:], rgs[g][:], ident[:])
                    if act_copy is True or sp_only or (act_copy and g in act_copy):
                        nc.scalar.mul(rows[:, 127:255], pt[:], 1.0)
                    else:
                        nc.vector.tensor_copy(rows[:, 127:255], pt[:])
                    nc.vector.tensor_copy(rows[:, 0:127], pt[:, 127:0:-1])
                src2 = AP(
                    tensor=rows.tensor,
                    offset=rows.offset + 127,
                    ap=[[256, gb], [-1, 128], [1, 128]],
                )
                dst2 = AP(
                    tensor=out,
                    offset=gstart[g] * L * L,
                    ap=[[L * L, gb], [L, 128], [1, 128]],
                )
                if sp_only or g >= ng - last_sp:
                    eng = nc.sync
                else:
                    eng = nc.sync if g % 2 == 0 else nc.scalar
                eng.dma_start(out=dst2, in_=src2)

            for g in range(ng):
                gb = groups[g]
                ps = ppool.tile([128, gb], f32)
                fa = fin_at if fin_at is not None else max(1, gb // 2)
                for j in range(gb):
                    if j == min(fa, gb - 1) and g >= 1:
                        finish(g - 1)
                    b = gstart[g] + j
                    s = b2s[b]
                    ht = htiles[s]
                    col = (b - sstart[s]) * W
                    for tp in range(T // 2):
                        off = col + K * tp
                        lhsT = AP(
                            tensor=ht.tensor,
                            offset=ht.offset + off,
                            ap=[[sgroups[s] * W, K], [P, 2], [1, 128]],
                        )
                        rhs = AP(
                            tensor=ht.tensor,
                            offset=ht.offset + off,
                            ap=[[sgroups[s] * W, K], [P, 2], [1, 1]],
                        )
                        nc.tensor.matmul(
                            ps[:, j : j + 1],
                            lhsT,
                            rhs,
                            start=(tp == 0),
                            stop=(tp == T // 2 - 1),
                            perf_mode=mybir.MatmulPerfMode.DoubleRow,
                        )

                rg = rpool.tile([128, gb], pal_dt if pal_mm else f32)
                if pal_mm:
                    nc.vector.tensor_copy(rg[:], ps[:])
                else:
                    nc.vector.tensor_scalar_mul(rg[:], ps[:], 1.0 / L)
                rgs[g] = rg
            finish(ng - 1)

    nc.compile()
    return nc


def kernel(inputs: np.ndarray) -> np.ndarray:
    global LAST_RESULT
    inputs = np.ascontiguousarray(np.asarray(inputs), dtype=np.float32)
    assert inputs.shape == (B, 2, N), inputs.shape

    if "nc" not in _CACHE:
        _CACHE["nc"] = build_nc()
    nc = _CACHE["nc"]

    k = np.arange(256)
    d = np.arange(128)
    spal = (d[:, None] == np.minimum(np.abs(k[None, :] - 127), 127)).astype(
        np.float32
    ) / L
    spal[:, 255] = 0.0
    in_maps = [
        {"inp": inputs[c * BPC : (c + 1) * BPC], "spald": spal}
        for c in range(NCORES)
    ]
    res = run_bass_kernel_spmd(nc, in_maps, list(range(NCORES)), trace=False)
    LAST_RESULT = res
    outf = np.concatenate([res.results[c]["out"] for c in range(NCORES)], axis=0)
    return outf.reshape(B, L, L, 1).astype(np.float32, copy=False)


# revision 25
# speedup vs baseline: 1.2563x; 1.0163x over previous
"""Trainium2 Bass kernel for nn_CovarianceLayer (Toeplitz-autocorrelation form).

Math: x = inputs[:,0,:] + i*inputs[:,1,:]  (B=256 complex signals, N=1024)
      cov[b,l,m] = Re(hankel @ hankel^H)[l,m] / L  with hankel[b,i,j] = x[b,(j+i)%N]
By circularity cov[b,l,m] = r_b[|l-m|] / L where
      r_b[d] = sum_n ( xr[n]xr[n+d] + xi[n]xi[n+d] )   (indices mod N)
i.e. each [L,L] output tile is a symmetric Toeplitz matrix fully
determined by a 128-lag autocorrelation r_b, so only r_b is computed.

Per-core pipeline (32 batches/core, pure data parallel):
  1. One gpsimd casting DMA per supergroup builds a doubled fp8e4m3 copy
     of x in DRAM (row per batch: [x0 x0 | x1 x1]; the duplication
     realizes the circular wrap). Separate DRAM tiles per supergroup
     keep the dependency tracking fine-grained.
  2. Per 8-batch supergroup, 2 HWDGE DMAs build a packed Hankel tile
     H[16c+p, j*W+u] = x_c[b_j, p+u] (overlapping-window access
     pattern; comps fp8-packed on partition halves).
  3. 32 DoubleRow fp8 matmuls per batch (each contracts K=32 offsets x
     comps times 2 k-tiles) accumulate r_b into one psum column; the
     moving operand is a single column of the same Hankel tile.
  4. Per compute group: DVE drains psum -> SBUF, then ONE f32r matmul
     against a constant palindrome matrix (spal[d,k] = [d==|k-127|]/L)
     applies transpose + mirror + 1/L in one PE op, and a single DVE
     copy stages the palindrome rows s_b[k] = r_b[|k-127|] in SBUF.
  5. One strided DMA per group expands the Toeplitz tiles straight from
     SBUF into the output: out[b,l,m] = s_b[127-l+m] (contiguous 512B
     runs on both sides, ~360GB/s on the DMA engines).
Group sizes taper so the first expansion DMA starts early and the last
group's drain->expand tail is short; expansion DMAs alternate SP/Act
queues and overlap later groups' matmuls.
"""

import numpy as np

import concourse.bacc as bacc
import concourse.mybir as mybir
import concourse.tile as tile
from concourse.bass_types import AP
from concourse.bass_utils import run_bass_kernel_spmd

B, L, N = 256, 128, 1024
NCORES = 8
BPC = B // NCORES  # 32 batches per core

P = 16  # n-offsets per chunk
K = 2 * P  # matmul contraction width
T = N // P  # 64 chunks per batch
W = N - P + 128  # 1136
CROW = 2 * N  # doubled per-comp row in xdup
ROW = 2 * CROW  # 4096 elems per batch

_CACHE = {}
LAST_RESULT = None

SGROUPS = [8, 8, 8, 8]  # hankel supergroup sizes
GROUPS = [6, 6, 6, 6, 4, 2, 2]  # compute/expansion group sizes
CAST_SPLIT = 0  # batches cast in the first (fast) casting DMA; 0 = single


def build_nc(sgroups=None, groups=None, cast_split=CAST_SPLIT, act_copy=False,
             warmup=False, ident_input=True, last_sp=0, sp_only=False,
             pal_mm="f32r", cgroups=None, fin_at=None, pbufs=(4, 4)):
    f8 = mybir.dt.float8e4
    f32 = mybir.dt.float32
    sgroups = list(SGROUPS if sgroups is None else sgroups)
    groups = list(GROUPS if groups is None else groups)
    assert sum(sgroups) == BPC and sum(groups) == BPC
    sstart = [sum(sgroups[:i]) for i in range(len(sgroups))]
    gstart = [sum(groups[:i]) for i in range(len(groups))]
    ng = len(groups)
    # map batch -> supergroup index
    b2s = []
    for s, sz in enumerate(sgroups):
        b2s += [s] * sz

    nc = bacc.Bacc(
        "TRN2", target_bir_lowering=False, debug=False, num_devices=NCORES
    )
    inp = nc.dram_tensor("inp", [BPC, 2, N], f32, kind="ExternalInput")
    pal_dt = {None: None, "f32": f32, "f32r": mybir.dt.float32r}[pal_mm]
    identd = (
        nc.dram_tensor("identd", [128, 128], f32, kind="ExternalInput")
        if ident_input and not pal_mm
        else None
    )
    spald = (
        nc.dram_tensor("spald", [128, 256], pal_dt, kind="ExternalInput")
        if pal_mm
        else None
    )
    out = nc.dram_tensor("out", [BPC, L, L], f32, kind="ExternalOutput")

    with tile.TileContext(nc) as tc:
        with (
            tc.tile_pool(name="const", bufs=1) as cpool,
            tc.tile_pool(name="dram", bufs=1, space="DRAM") as dpool,
            tc.tile_pool(name="hank", bufs=len(sgroups)) as hpool,
            tc.tile_pool(name="spal", bufs=ng) as spool,
            tc.tile_pool(name="rr", bufs=ng) as rpool,
            tc.tile_pool(name="psum", bufs=pbufs[0], space="PSUM") as ppool,
            tc.tile_pool(name="pst", bufs=pbufs[1], space="PSUM") as tpool,
        ):
            # --- doubled fp8 signal in DRAM via per-supergroup casting
            # DMAs, emitted FIRST. Each supergroup gets its own DRAM tile so
            # its hankel DMAs depend only on its own cast (tile-granular
            # dependency tracking). ---
            flat = inp[:].rearrange("b c n -> (b c) n")
            cgs = list(cgroups) if cgroups else list(sgroups)
            assert sum(cgs) == BPC
            cstart = [sum(cgs[:i]) for i in range(len(cgs))]
            xdups = []  # one per cast group
            for si, ssz in enumerate(cgs):
                xd = dpool.tile([2 * ssz, CROW], f8)
                xdups.append(xd)
                r0 = 2 * cstart[si]
                src0 = AP(
                    tensor=flat.tensor,
                    offset=flat.offset + r0 * N,
                    ap=[[N, 2 * ssz], [0, 2], [1, N]],
                )
                dst0 = AP(
                    tensor=xd.tensor,
                    offset=xd.offset,
                    ap=[[CROW, 2 * ssz], [N, 2], [1, N]],
                )
                nc.gpsimd.dma_start(out=dst0, in_=src0)

            # --- identity for PE transpose (needed only ~9us in) ---
            if pal_mm:
                spal_t = cpool.tile([128, 256], pal_dt)
                (nc.sync if sp_only else nc.scalar).dma_start(
                    out=spal_t[:], in_=spald[:]
                )
                ident = None
            else:
                ident = cpool.tile([128, 128], f32)
                if ident_input:
                    (nc.sync if sp_only else nc.scalar).dma_start(
                        out=ident[:], in_=identd[:]
                    )
                else:
                    ones = cpool.tile([128, 128], f32)
                    nc.vector.memset(ones[:], 1.0)
                    nc.gpsimd.affine_select(
                        out=ident[:],
                        in_=ones[:],
                        pattern=[[1, 128]],
                        compare_op=mybir.AluOpType.is_equal,
                        fill=0.0,
                        base=0,
                        channel_multiplier=-1,
                    )
            if warmup:
                warm = cpool.tile([1, 2], f32)
                nc.vector.memset(warm[:], 1.0)
                nc.scalar.mul(warm[:], warm[:], 1.0)

            # --- hankel tiles: H[16c+p, j*W+u] = x_c[b_j, p+u] ---
            htiles = []
            for s, ssz in enumerate(sgroups):
                ht = hpool.tile([K, ssz * W], f8)
                htiles.append(ht)
                ci = max(i for i in range(len(cgs)) if cstart[i] <= sstart[s])
                assert cstart[ci] + cgs[ci] >= sstart[s] + ssz, "sg spans casts"
                xd = xdups[ci]
                xoff = 2 * (sstart[s] - cstart[ci]) * CROW
                if ssz == 1:
                    src = AP(
                        tensor=xd.tensor,
                        offset=xd.offset + xoff,
                        ap=[[CROW, 2], [1, P], [1, W]],
                    )
                    eng = nc.sync if (sp_only or s % 2 == 0) else nc.scalar
                    eng.dma_start(out=ht[:], in_=src)
                else:
                    for c in range(2):
                        src = AP(
                            tensor=xd.tensor,
                            offset=xd.offset + xoff + c * CROW,
                            ap=[[1, P], [ROW, ssz], [1, W]],
                        )
                        eng = nc.sync if (sp_only or c == 0) else nc.scalar
                        eng.dma_start(out=ht[P * c : P * c + P, :], in_=src)

            rgs = {}

            def finish(g):
                gb = groups[g]
                rows = spool.tile([gb, 256], f32)
                if pal_mm:
                    # one matmul applies transpose+mirror+1/L:
                    # pt2[j,k] = sum_d rg[d,j]*spal[d,k],  spal[d,k]=[d==|k-127|]/L
                    pt2 = tpool.tile([gb, 256], f32)
                    nc.tensor.matmul(pt2[:], rgs[g][:], spal_t[:])
                    nc.vector.tensor_copy(rows[:, 0:255], pt2[:, 0:255])
                else:
                    pt = tpool.tile([gb, 128], f32)
                    nc.tensor.transpose(pt[:], rgs[g][:], ident[:])
                    if act_copy is True or sp_only or (act_copy and g in act_copy):
                        nc.scalar.mul(rows[:, 127:255], pt[:], 1.0)
                    else:
                        nc.vector.tensor_copy(rows[:, 127:255], pt[:])
                    nc.vector.tensor_copy(rows[:, 0:127], pt[:, 127:0:-1])
                src2 = AP(
                    tensor=rows.tensor,
                    offset=rows.offset + 127,
                    ap=[[256, gb], [-1, 128], [1, 128]],
                )
                dst2 = AP(
                    tensor=out,
                    offset=gstart[g] * L * L,
                    ap=[[L * L, gb], [L, 128], [1, 128]],
                )
                if sp_only or g >= ng - last_sp:
                    eng = nc.sync
                else:
                    eng = nc.sync if g % 2 == 0 else nc.scalar
                eng.dma_start(out=dst2, in_=src2)

            for g in range(ng):
                gb = groups[g]
                ps = ppool.tile([128, gb], f32)
                fa = fin_at if fin_at is not None else max(1, gb // 2)
                for j in range(gb):
                    if j == min(fa, gb - 1) and g >= 1:
                        finish(g - 1)
                    b = gstart[g] + j
                    s = b2s[b]
                    ht = htiles[s]
                    col = (b - sstart[s]) * W
                    for tp in range(T // 2):
                        off = col + K * tp
                        lhsT = AP(
                            tensor=ht.tensor,
                            offset=ht.offset + off,
                            ap=[[sgroups[s] * W, K], [P, 2], [1, 128]],
                        )
                        rhs = AP(
                            tensor=ht.tensor,
                            offset=ht.offset + off,
                            ap=[[sgroups[s] * W, K], [P, 2], [1, 1]],
                        )
                        nc.tensor.matmul(
                            ps[:, j : j + 1],
                            lhsT,
                            rhs,
                            start=(tp == 0),
                            stop=(tp == T // 2 - 1),
                            perf_mode=mybir.MatmulPerfMode.DoubleRow,
                        )

                rg = rpool.tile([128, gb], pal_dt if pal_mm else f32)
                if pal_mm:
                    nc.vector.tensor_copy(rg[:], ps[:])
                else:
                    nc.vector.tensor_scalar_mul(rg[:], ps[:], 1.0 / L)
                rgs[g] = rg
            finish(ng - 1)

    nc.compile()
    return nc


def kernel(inputs: np.ndarray) -> np.ndarray:
    global LAST_RESULT
    inputs = np.ascontiguousarray(np.asarray(inputs), dtype=np.float32)
    assert inputs.shape == (B, 2, N), inputs.shape

    if "nc" not in _CACHE:
        _CACHE["nc"] = build_nc()
    nc = _CACHE["nc"]

    k = np.arange(256)
    d = np.arange(128)
    spal = (d[:, None] == np.minimum(np.abs(k[None, :] - 127), 127)).astype(
        np.float32
    ) / L
    spal[:, 255] = 0.0
    in_maps = [
        {"inp": inputs[c * BPC : (c + 1) * BPC], "spald": spal}
        for c in range(NCORES)
    ]
    res = run_bass_kernel_spmd(nc, in_maps, list(range(NCORES)), trace=False)
    LAST_RESULT = res
    outf = np.concatenate([res.results[c]["out"] for c in range(NCORES)], axis=0)
    return outf.reshape(B, L, L, 1).astype(np.float32, copy=False)
